# revision 11
# baseline (speedup 1.0000x reference)
"""Trainium2 Bass kernel for the mhe-embedding-lookup + projection problem.

Strategy (8 NeuronCores, pure data-parallel over tokens):
  - Host: compute the 16 per-head table row indices per token (int64 modular
    arithmetic does not fit int32 on device), cast the embedding table and
    projection weights to bf16, pre-transpose the weights.
  - Device (per core, 1024 of the 8192 tokens):
      * indirect-DMA gather of 16 x 64 bf16 table rows per token (token-major)
      * PE transpose to feature-major (contraction layout)
      * 5 matmuls (value + 4 keys) vs bf16 weights, f32 PSUM accumulation
      * bias add (f32), rmsnorm for the keys (f32), f32 output store
  - Host: concatenate the 8 token shards into the full (5,4,2048,1024) output.
"""

import numpy as np
import ml_dtypes

import concourse.bass as bass
import concourse.bacc as bacc
import concourse.mybir as mybir
import concourse.tile as tile
from concourse.bass import IndirectOffsetOnAxis
from concourse.bass_utils import run_bass_kernel_spmd
from concourse.masks import make_identity

# ---- problem constants (hardcoded; kernel.py must be self-contained) ----
B, S = 4, 2048
HID = 1024            # output hidden dim per projection
NHEAD = 8             # heads per ngram order
D = 64                # embed dim per head
HC = 4                # number of key projections
PAD_ID = 2
N_TOTAL = 4138152     # total rows in the shared embedding table (sum of 16 head primes)
ENG_H = 1024          # 16 heads * 64
N_CORES = 8
T = (B * S) // N_CORES          # tokens per core = 1024
JT = T // 128                   # token tiles per core = 8
EC = ENG_H // 128               # feature chunks = 8
NO = 1 + HC                     # outputs: value + 4 keys = 5
EPS = float(np.finfo(np.float32).eps)

F32 = mybir.dt.float32
BF16 = mybir.dt.bfloat16
I32 = mybir.dt.int32


def build_nc():
    """Build the per-core Bass program (identical on all 8 cores)."""
    nc = bacc.Bacc(None, target_bir_lowering=False)

    table_t = nc.dram_tensor("table", [N_TOTAL, D], BF16, kind="ExternalInput")
    idx_t = nc.dram_tensor("idx", [128, JT, 16], I32, kind="ExternalInput")
    wt_t = nc.dram_tensor("wT", [128, EC, NO, HID], BF16, kind="ExternalInput")
    bias_t = nc.dram_tensor("bias", [NO, HID], F32, kind="ExternalInput")
    out_t = nc.dram_tensor("out", [NO, T, HID], F32, kind="ExternalOutput")

    with tile.TileContext(nc) as tc:
        with (
            tc.tile_pool(name="consts", bufs=1) as consts,
            tc.tile_pool(name="feat", bufs=3) as featp,
            tc.tile_pool(name="featT", bufs=3) as featTp,
            tc.tile_pool(name="stage", bufs=6) as stagep,
            tc.tile_pool(name="scratch", bufs=2) as scratchp,
            tc.tile_pool(name="small", bufs=8) as smallp,
            tc.tile_pool(name="psum", bufs=4, space="PSUM") as psump,
        ):
            # ---- load constants ----
            idx_sb = consts.tile([128, JT, 16], I32)
            nc.sync.dma_start(out=idx_sb[:], in_=idx_t[:])

            wt_sb = consts.tile([128, EC, NO, HID], BF16)
            nc.sync.dma_start(out=wt_sb[:], in_=wt_t[:])

            # bias broadcast to all 128 partitions via a step-0 DMA read
            bias_sb = consts.tile([128, NO, HID], F32)
            bias_bcast = bass.AP(
                tensor=bias_t, offset=0, ap=[[0, 128], [1, NO * HID]]
            )
            nc.sync.dma_start(out=bias_sb[:], in_=bias_bcast)

            eps_sb = consts.tile([128, 1], F32)
            nc.vector.memset(eps_sb[:], EPS)

            # output groups sized to fit PSUM (psum pool = 3 tiles x 2 banks,
            # transpose pool 2 banks -> 8 banks total)
            groups = [(0, 1, 2), (3, 4)]

            for j in range(JT):
                # ---- gather: 128 tokens x 16 heads x 64 bf16 ----
                # HW vector-indirect DMA semantics: ONE index per dest
                # partition row, each moving dest-free-size contiguous
                # elements. So one call per head: 128 rows of 64 bf16.
                feat = featp.tile([128, ENG_H], BF16)
                for h in range(16):
                    nc.gpsimd.indirect_dma_start(
                        out=feat[:, h * D:(h + 1) * D],
                        out_offset=None,
                        in_=table_t[:],
                        in_offset=IndirectOffsetOnAxis(
                            ap=idx_sb[:, j, h:h + 1], axis=0
                        ),
                    )

                # ---- transpose to feature-major via DMA xbar (off the PE) ----
                featT = featTp.tile([128, EC, 128], BF16)
                for ec in range(EC):
                    nc.sync.dma_start(
                        out=featT[:, ec, :],
                        in_=feat[:, ec * 128:(ec + 1) * 128],
                        transpose=True,
                    )

                # ---- matmuls + epilogues ----
                for grp in groups:
                    ps = {}
                    for o in grp:
                        ps[o] = psump.tile([128, HID], F32, name="ps", tag="ps")
                    for ec in range(EC):
                        for o in grp:
                            for b in range(2):
                                nc.tensor.matmul(
                                    out=ps[o][:, b * 512:(b + 1) * 512],
                                    lhsT=featT[:, ec, :],
                                    rhs=wt_sb[:, ec, o, b * 512:(b + 1) * 512],
                                    start=(ec == 0),
                                    stop=(ec == EC - 1),
                                )
                    for o in grp:
                        x = stagep.tile([128, HID], F32)
                        nc.vector.tensor_tensor(
                            out=x[:], in0=ps[o][:], in1=bias_sb[:, o, :],
                            op=mybir.AluOpType.add,
                        )
                        if o == 0:
                            nc.sync.dma_start(
                                out=out_t[0, j * 128:(j + 1) * 128, :], in_=x[:]
                            )
                        else:
                            sq = scratchp.tile([128, HID], F32)
                            ssq = smallp.tile([128, 1], F32)
                            nc.scalar.activation(
                                out=sq[:], in_=x[:],
                                func=mybir.ActivationFunctionType.Square,
                                accum_out=ssq[:],
                            )
                            s = smallp.tile([128, 1], F32)
                            nc.scalar.activation(
                                out=s[:], in_=ssq[:],
                                func=mybir.ActivationFunctionType.Sqrt,
                                bias=eps_sb[:], scale=1.0 / ENG_H,
                            )
                            r = smallp.tile([128, 1], F32)
                            nc.vector.reciprocal(r[:], s[:])
                            ox = stagep.tile([128, HID], F32)
                            nc.vector.tensor_scalar_mul(ox[:], x[:], r[:])
                            nc.sync.dma_start(
                                out=out_t[o, j * 128:(j + 1) * 128, :], in_=ox[:]
                            )

    nc.compile()
    return nc


# ---- host-side input preparation ----

def compute_indices(input_ids, multipliers, vocab_sizes, offsets):
    """(B,S) int64 ids -> (B*S, 16) int32 *effective* table row indices.

    Mirrors reference._gather_features exactly using the ambient jax
    (including any environment-level monkey patches of %), and resolves
    jnp's out-of-bounds/negative index semantics by gathering an arange.
    """
    import jax
    import jax.numpy as jnp
    jax.config.update("jax_enable_x64", True)

    with jax.default_device(jax.devices("cpu")[0]):
        ids = jnp.asarray(np.asarray(input_ids))
        mult = jnp.asarray(np.asarray(multipliers))
        vs = jnp.asarray(np.asarray(vocab_sizes))
        off = jnp.asarray(np.asarray(offsets))
        rows = jnp.arange(N_TOTAL, dtype=jnp.int32)

        def shift_right(a, k):
            if k == 0:
                return a
            pad = jnp.full((a.shape[0], k), PAD_ID, dtype=a.dtype)
            return jnp.concatenate([pad, a[:, :-k]], axis=1)

        effs = []
        for n in range(2, 4):          # ngram orders 2 and 3
            mix = jnp.zeros_like(ids)
            for k in range(n):
                mix = mix + shift_right(ids, k) * mult[k]
            h0 = (n - 2) * NHEAD
            v = vs[h0:h0 + NHEAD]
            o = off[h0:h0 + NHEAD]
            idx = o[None, None, :] + (mix[..., None] % v[None, None, :])
            effs.append(rows[idx])     # jnp take semantics (clip/wrap)
        eff = jnp.concatenate(effs, axis=-1)          # (B, S, 16)
    return np.asarray(eff).reshape(-1, 16).astype(np.int32)


def prepare_in_maps(inputs):
    idx_flat = compute_indices(
        inputs["input_ids"], inputs["multipliers"],
        inputs["vocab_sizes"], inputs["offsets"],
    )
    # per core c, partition p, tok-tile j, head h: token 1024c + 128j + p
    idx_all = idx_flat.reshape(N_CORES, JT, 128, 16).transpose(0, 2, 1, 3)
    idx_all = np.ascontiguousarray(idx_all)

    table_bf16 = np.asarray(inputs["emb_table"], dtype=np.float32).astype(
        ml_dtypes.bfloat16
    )

    W = np.concatenate(
        [np.asarray(inputs["value_w"], np.float32)[None],
         np.asarray(inputs["key_w"], np.float32)], axis=0
    )  # (5, HID, ENG_H) indexed [o, h, e]
    wT = W.transpose(2, 0, 1).reshape(EC, 128, NO, HID).transpose(1, 0, 2, 3)
    wT = np.ascontiguousarray(wT).astype(ml_dtypes.bfloat16)

    bias = np.concatenate(
        [np.asarray(inputs["value_b"], np.float32)[None],
         np.asarray(inputs["key_b"], np.float32)], axis=0
    ).astype(np.float32)  # (5, HID)

    return [
        {"table": table_bf16, "idx": idx_all[c], "wT": wT, "bias": bias}
        for c in range(N_CORES)
    ]


def assemble_output(results, norm_w):
    full = np.empty((NO, B * S, HID), dtype=np.float32)
    for c in range(N_CORES):
        full[:, c * T:(c + 1) * T, :] = results[c]["out"]
    full = full.reshape(NO, B, S, HID)
    nw = np.asarray(norm_w, dtype=np.float32)
    if not np.all(nw == 1.0):
        full[1:] *= nw[:, None, None, :]
    return full


_NC_CACHE = {}


def get_nc():
    if "nc" not in _NC_CACHE:
        _NC_CACHE["nc"] = build_nc()
    return _NC_CACHE["nc"]


def kernel(**inputs):
    nc = get_nc()
    in_maps = prepare_in_maps(inputs)
    res = run_bass_kernel_spmd(nc, in_maps, core_ids=list(range(N_CORES)))
    return assemble_output(res.results, inputs["norm_w"])


# revision 17
# speedup vs baseline: 1.2446x; 1.2446x over previous
"""Trainium2 Bass kernel for the mhe-embedding-lookup + projection problem.

Strategy (8 NeuronCores, pure data-parallel over tokens):
  - Host: compute the 16 per-head table row indices per token (int64 modular
    arithmetic does not fit int32 on device), cast the embedding table and
    projection weights to bf16, pre-transpose the weights.
  - Device (per core, 1024 of the 8192 tokens):
      * indirect-DMA gather of 16 x 64 bf16 table rows per token (token-major)
      * PE transpose to feature-major (contraction layout)
      * 5 matmuls (value + 4 keys) vs bf16 weights, f32 PSUM accumulation
      * bias add (f32), rmsnorm for the keys (f32), f32 output store
  - Host: concatenate the 8 token shards into the full (5,4,2048,1024) output.
"""

import numpy as np
import ml_dtypes

import concourse.bass as bass
import concourse.bacc as bacc
import concourse.mybir as mybir
import concourse.tile as tile
from concourse.bass import IndirectOffsetOnAxis
from concourse.bass_utils import run_bass_kernel_spmd
from concourse.masks import make_identity

# ---- problem constants (hardcoded; kernel.py must be self-contained) ----
B, S = 4, 2048
HID = 1024            # output hidden dim per projection
NHEAD = 8             # heads per ngram order
D = 64                # embed dim per head
HC = 4                # number of key projections
PAD_ID = 2
N_TOTAL = 4138152     # total rows in the shared embedding table (sum of 16 head primes)
ENG_H = 1024          # 16 heads * 64
N_CORES = 8
T = (B * S) // N_CORES          # tokens per core = 1024
JT = T // 128                   # token tiles per core = 8
EC = ENG_H // 128               # feature chunks = 8
NO = 1 + HC                     # outputs: value + 4 keys = 5
EPS = float(np.finfo(np.float32).eps)

F32 = mybir.dt.float32
BF16 = mybir.dt.bfloat16
I32 = mybir.dt.int32


def build_nc():
    """Build the per-core Bass program (identical on all 8 cores)."""
    nc = bacc.Bacc(None, target_bir_lowering=False)

    table_t = nc.dram_tensor("table", [N_TOTAL, D], BF16, kind="ExternalInput")
    idx_t = nc.dram_tensor("idx", [128, JT, 16], I32, kind="ExternalInput")
    wt_t = nc.dram_tensor("wT", [128, EC, NO, HID], BF16, kind="ExternalInput")
    bias_t = nc.dram_tensor("bias", [NO, HID], F32, kind="ExternalInput")
    out_t = nc.dram_tensor("out", [NO, T, HID], F32, kind="ExternalOutput")

    with tile.TileContext(nc) as tc:
        with (
            tc.tile_pool(name="consts", bufs=1) as consts,
            tc.tile_pool(name="feat", bufs=3) as featp,
            tc.tile_pool(name="featT", bufs=3) as featTp,
            tc.tile_pool(name="stage", bufs=6) as stagep,
            tc.tile_pool(name="scratch", bufs=2) as scratchp,
            tc.tile_pool(name="small", bufs=8) as smallp,
            tc.tile_pool(name="psum", bufs=3, space="PSUM") as psump,
            tc.tile_pool(name="tpsum", bufs=2, space="PSUM") as tpsump,
        ):
            # ---- load constants ----
            idx_sb = consts.tile([128, JT, 16], I32)
            nc.sync.dma_start(out=idx_sb[:], in_=idx_t[:])

            # load weights in per-ec chunks so the first matmuls don't wait
            # for the whole 10.5 MB transfer
            wt_sb = consts.tile([128, EC, NO, HID], BF16)
            for ec in range(EC):
                nc.sync.dma_start(out=wt_sb[:, ec, :, :], in_=wt_t[:, ec, :, :])

            # bias broadcast to all 128 partitions via a step-0 DMA read
            bias_sb = consts.tile([128, NO, HID], F32)
            bias_bcast = bass.AP(
                tensor=bias_t, offset=0, ap=[[0, 128], [1, NO * HID]]
            )
            nc.sync.dma_start(out=bias_sb[:], in_=bias_bcast)

            ident = consts.tile([128, 128], BF16)
            make_identity(nc, ident[:])

            eps_sb = consts.tile([128, 1], F32)
            nc.vector.memset(eps_sb[:], EPS)

            # output groups sized to fit PSUM (psum pool = 3 tiles x 2 banks,
            # transpose pool 2 banks -> 8 banks total)
            groups = [(0, 1, 2), (3, 4)]

            for j in range(JT):
                # ---- gather: 128 tokens x 16 heads x 64 bf16 ----
                # HW vector-indirect DMA semantics: ONE index per dest
                # partition row, each moving dest-free-size contiguous
                # elements. So one call per head: 128 rows of 64 bf16.
                feat = featp.tile([128, ENG_H], BF16)
                for h in range(16):
                    nc.gpsimd.indirect_dma_start(
                        out=feat[:, h * D:(h + 1) * D],
                        out_offset=None,
                        in_=table_t[:],
                        in_offset=IndirectOffsetOnAxis(
                            ap=idx_sb[:, j, h:h + 1], axis=0
                        ),
                    )

                # ---- transpose to feature-major via PE identity matmul ----
                featT = featTp.tile([128, EC, 128], BF16)
                for ec in range(EC):
                    tp = tpsump.tile([128, 128], BF16)
                    nc.tensor.transpose(
                        out=tp[:],
                        in_=feat[:, ec * 128:(ec + 1) * 128],
                        identity=ident[:],
                    )
                    nc.scalar.copy(featT[:, ec, :], tp[:])

                # ---- matmuls + epilogues ----
                for grp in groups:
                    ps = {}
                    for o in grp:
                        ps[o] = psump.tile([128, HID], F32, name="ps", tag="ps")
                    for ec in range(EC):
                        for o in grp:
                            for b in range(2):
                                nc.tensor.matmul(
                                    out=ps[o][:, b * 512:(b + 1) * 512],
                                    lhsT=featT[:, ec, :],
                                    rhs=wt_sb[:, ec, o, b * 512:(b + 1) * 512],
                                    start=(ec == 0),
                                    stop=(ec == EC - 1),
                                )
                    for o in grp:
                        x = stagep.tile([128, HID], F32)
                        nc.vector.tensor_tensor(
                            out=x[:], in0=ps[o][:], in1=bias_sb[:, o, :],
                            op=mybir.AluOpType.add,
                        )
                        if o == 0:
                            nc.sync.dma_start(
                                out=out_t[0, j * 128:(j + 1) * 128, :], in_=x[:]
                            )
                        else:
                            sq = scratchp.tile([128, HID], F32)
                            ssq = smallp.tile([128, 1], F32)
                            nc.scalar.activation(
                                out=sq[:], in_=x[:],
                                func=mybir.ActivationFunctionType.Square,
                                accum_out=ssq[:],
                            )
                            s = smallp.tile([128, 1], F32)
                            nc.scalar.activation(
                                out=s[:], in_=ssq[:],
                                func=mybir.ActivationFunctionType.Sqrt,
                                bias=eps_sb[:], scale=1.0 / ENG_H,
                            )
                            r = smallp.tile([128, 1], F32)
                            nc.vector.reciprocal(r[:], s[:])
                            ox = stagep.tile([128, HID], F32)
                            nc.vector.tensor_scalar_mul(ox[:], x[:], r[:])
                            nc.sync.dma_start(
                                out=out_t[o, j * 128:(j + 1) * 128, :], in_=ox[:]
                            )

    nc.compile()
    return nc


# ---- host-side input preparation ----

def compute_indices(input_ids, multipliers, vocab_sizes, offsets):
    """(B,S) int64 ids -> (B*S, 16) int32 *effective* table row indices.

    Mirrors reference._gather_features exactly using the ambient jax
    (including any environment-level monkey patches of %), and resolves
    jnp's out-of-bounds/negative index semantics by gathering an arange.
    """
    import jax
    import jax.numpy as jnp
    jax.config.update("jax_enable_x64", True)

    with jax.default_device(jax.devices("cpu")[0]):
        ids = jnp.asarray(np.asarray(input_ids))
        mult = jnp.asarray(np.asarray(multipliers))
        vs = jnp.asarray(np.asarray(vocab_sizes))
        off = jnp.asarray(np.asarray(offsets))
        rows = jnp.arange(N_TOTAL, dtype=jnp.int32)

        def shift_right(a, k):
            if k == 0:
                return a
            pad = jnp.full((a.shape[0], k), PAD_ID, dtype=a.dtype)
            return jnp.concatenate([pad, a[:, :-k]], axis=1)

        effs = []
        for n in range(2, 4):          # ngram orders 2 and 3
            mix = jnp.zeros_like(ids)
            for k in range(n):
                mix = mix + shift_right(ids, k) * mult[k]
            h0 = (n - 2) * NHEAD
            v = vs[h0:h0 + NHEAD]
            o = off[h0:h0 + NHEAD]
            idx = o[None, None, :] + (mix[..., None] % v[None, None, :])
            effs.append(rows[idx])     # jnp take semantics (clip/wrap)
        eff = jnp.concatenate(effs, axis=-1)          # (B, S, 16)
    return np.asarray(eff).reshape(-1, 16).astype(np.int32)


def prepare_in_maps(inputs):
    idx_flat = compute_indices(
        inputs["input_ids"], inputs["multipliers"],
        inputs["vocab_sizes"], inputs["offsets"],
    )
    # per core c, partition p, tok-tile j, head h: token 1024c + 128j + p
    idx_all = idx_flat.reshape(N_CORES, JT, 128, 16).transpose(0, 2, 1, 3)
    idx_all = np.ascontiguousarray(idx_all)

    table_bf16 = np.asarray(inputs["emb_table"], dtype=np.float32).astype(
        ml_dtypes.bfloat16
    )

    W = np.concatenate(
        [np.asarray(inputs["value_w"], np.float32)[None],
         np.asarray(inputs["key_w"], np.float32)], axis=0
    )  # (5, HID, ENG_H) indexed [o, h, e]
    wT = W.transpose(2, 0, 1).reshape(EC, 128, NO, HID).transpose(1, 0, 2, 3)
    wT = np.ascontiguousarray(wT).astype(ml_dtypes.bfloat16)

    bias = np.concatenate(
        [np.asarray(inputs["value_b"], np.float32)[None],
         np.asarray(inputs["key_b"], np.float32)], axis=0
    ).astype(np.float32)  # (5, HID)

    return [
        {"table": table_bf16, "idx": idx_all[c], "wT": wT, "bias": bias}
        for c in range(N_CORES)
    ]


def assemble_output(results, norm_w):
    full = np.empty((NO, B * S, HID), dtype=np.float32)
    for c in range(N_CORES):
        full[:, c * T:(c + 1) * T, :] = results[c]["out"]
    full = full.reshape(NO, B, S, HID)
    nw = np.asarray(norm_w, dtype=np.float32)
    if not np.all(nw == 1.0):
        full[1:] *= nw[:, None, None, :]
    return full


_NC_CACHE = {}


def _enable_ldw_opt():
    """Let walrus elide redundant LDWEIGHTS (consecutive matmuls here share
    the same stationary operand 6x / 4x in a row)."""
    import concourse.bass_utils as bu
    if getattr(bu.run_command, "_ldw_patched", False):
        return
    orig = bu.run_command

    def patched(argv, **kwargs):
        argv = ["--enable-ldw-opt=true" if a == "--enable-ldw-opt=false" else a
                for a in argv]
        return orig(argv, **kwargs)

    patched._ldw_patched = True
    bu.run_command = patched


def get_nc():
    if "nc" not in _NC_CACHE:
        _NC_CACHE["nc"] = build_nc()
    return _NC_CACHE["nc"]


def kernel(**inputs):
    nc = get_nc()
    in_maps = prepare_in_maps(inputs)
    res = run_bass_kernel_spmd(nc, in_maps, core_ids=list(range(N_CORES)))
    return assemble_output(res.results, inputs["norm_w"])
